# revision 4
# baseline (speedup 1.0000x reference)
"""Trainium2 Bass kernel for DCRN fusion (gated combine + sparse message passing + residual).

    z_i = a*z1 + b*z2                                  [N, D]
    z_l[r] = sum_{e: row[e]==r} val[e] * z_i[col[e]]   [N, D]
    out = alpha*z_l + (1-alpha)*z_i

Sharding: dest-rows are partitioned across 8 NeuronCores in 128-row blocks
(49 blocks/core). Each core computes the full z_i table in bf16 from
replicated bf16 inputs (message path), gathers source rows per edge with
SWDGE dma_gather, and does the per-block segment-sum on the PE via
val-scaled one-hot selection matrices accumulated in PSUM. The residual
path uses exact f32 own-shard inputs.

Self-contained: hardcodes the reference shapes/dtypes; all index-space
preprocessing (bucketing/sorting/padding of the edge list) is host-side
numpy inside kernel().
"""

import os
import numpy as np
import ml_dtypes

import concourse.bacc as bacc
import concourse.mybir as mybir
import concourse.tile as tile

P = 128
N_CORES = 8
D = 128

BF16 = mybir.dt.bfloat16
F32 = mybir.dt.float32
I16 = mybir.dt.int16

CALL_CH = 8           # gather chunks (of 128 idxs) per dma_gather call; 1024 descs fits the SWDGE ring
NQ = 4                # SWDGE queues

# exposed for the test harness
_LAST_RESULTS = None
_TRACE = os.environ.get("GNN_TRACE", "0") == "1"
_SIM = os.environ.get("GNN_SIM", "0") == "1"


def _host_prep(z1, z2, adj_row, adj_col, adj_val, a, b):
    """Bucket/sort/pad the edge list; build per-core input arrays."""
    N = z1.shape[0]
    n_blocks_total = -(-N // P)                   # 391
    blocks_per_core = -(-n_blocks_total // N_CORES)  # 49
    rows_per_core = blocks_per_core * P           # 6272
    n_src_pad = n_blocks_total * P                # 50048
    split = n_src_pad // 2                        # 25024 (< 32768)

    bf = ml_dtypes.bfloat16
    blk = adj_row // P
    is_hi = (adj_col >= split).astype(np.int64)
    order = np.lexsort((adj_col, is_hi, blk))
    d_s = adj_row[order]
    c_s = adj_col[order]
    v_s = adj_val[order]
    h_s = is_hi[order]
    b_s = blk[order]

    key = b_s * 2 + h_s
    n_groups = n_blocks_total * 2
    cnt = np.bincount(key, minlength=n_groups)
    grp_start = np.concatenate([[0], np.cumsum(cnt)])[:-1]
    rank = np.arange(len(order)) - grp_start[key]

    cnt2 = cnt.reshape(n_blocks_total, 2)
    C_lo = max(1, int(-(-cnt2[:, 0].max() // P)))
    C_hi = max(1, int(-(-cnt2[:, 1].max() // P)))
    T_lo = blocks_per_core * C_lo
    T_hi = blocks_per_core * C_hi

    core_s = b_s // blocks_per_core
    lblk_s = b_s % blocks_per_core

    idx_lo = np.zeros((N_CORES, T_lo * P), np.int16)
    val_lo = np.zeros((N_CORES, T_lo * P), np.float32)
    slot_lo = np.zeros((N_CORES, T_lo * P), np.float32)
    idx_hi = np.zeros((N_CORES, T_hi * P), np.int16)
    val_hi = np.zeros((N_CORES, T_hi * P), np.float32)
    slot_hi = np.zeros((N_CORES, T_hi * P), np.float32)

    m = h_s == 0
    pos = lblk_s[m] * (C_lo * P) + rank[m]
    idx_lo[core_s[m], pos] = c_s[m].astype(np.int16)
    val_lo[core_s[m], pos] = v_s[m]
    slot_lo[core_s[m], pos] = (d_s[m] % P).astype(np.float32)
    m = h_s == 1
    pos = lblk_s[m] * (C_hi * P) + rank[m]
    idx_hi[core_s[m], pos] = (c_s[m] - split).astype(np.int16)
    val_hi[core_s[m], pos] = v_s[m]
    slot_hi[core_s[m], pos] = (d_s[m] % P).astype(np.float32)

    def wrap16(x):
        # [n] -> [128, n//16]; slot i -> [i%16, i//16], replicated across 8 gpsimd cores
        n = x.shape[-1]
        w = x.reshape(-1, n // 16, 16)
        w = np.swapaxes(w, -1, -2)  # [cores?, 16, n//16]
        return np.tile(w, (1, 8, 1))

    def meta(x, t):
        # [T*P] -> [128, T] column t = chunk t
        return np.ascontiguousarray(x.reshape(-1, t, P).swapaxes(-1, -2))

    # replicated bf16 full inputs (zero-padded to n_src_pad rows)
    def pad_bf(x):
        out = np.zeros((n_src_pad, D), bf)
        out[:N] = x.astype(bf)
        return out

    # own-shard f32 inputs (zero-padded)
    def own(x, c):
        out = np.zeros((rows_per_core, D), np.float32)
        lo = c * rows_per_core
        hi = min(N, lo + rows_per_core)
        if hi > lo:
            out[: hi - lo] = x[lo:hi]
        return out

    z1b, z2b, ab, bb = pad_bf(z1), pad_bf(z2), pad_bf(a), pad_bf(b)
    iota = np.tile(np.arange(P, dtype=np.float32)[None, :], (P, 1)).astype(bf)

    idx_lo_w = wrap16(idx_lo).astype(np.int16)
    idx_hi_w = wrap16(idx_hi).astype(np.int16)

    in_maps = []
    for c in range(N_CORES):
        in_maps.append({
            "z1b": z1b, "z2b": z2b, "ab": ab, "bb": bb,
            "z1o": own(z1, c), "z2o": own(z2, c),
            "ao": own(a, c), "bo": own(b, c),
            "idx_lo": idx_lo_w[c], "idx_hi": idx_hi_w[c],
            "dest_lo": meta(slot_lo[c], T_lo), "val_lo": meta(val_lo[c], T_lo),
            "dest_hi": meta(slot_hi[c], T_hi), "val_hi": meta(val_hi[c], T_hi),
            "iota": iota,
        })

    cfg = dict(
        N=N, n_src_pad=n_src_pad, split=split,
        blocks_per_core=blocks_per_core, rows_per_core=rows_per_core,
        C_lo=C_lo, C_hi=C_hi, T_lo=T_lo, T_hi=T_hi,
    )
    return in_maps, cfg


def _build_program(cfg, alpha):
    n_src_pad = cfg["n_src_pad"]
    split = cfg["split"]
    NB = cfg["blocks_per_core"]
    RPC = cfg["rows_per_core"]
    C_lo, C_hi = cfg["C_lo"], cfg["C_hi"]
    T_lo, T_hi = cfg["T_lo"], cfg["T_hi"]

    nc = bacc.Bacc("TRN2", target_bir_lowering=False, debug=False,
                   num_swdge_queues=NQ, num_devices=N_CORES)

    z1b = nc.dram_tensor("z1b", [n_src_pad, D], BF16, kind="ExternalInput")
    z2b = nc.dram_tensor("z2b", [n_src_pad, D], BF16, kind="ExternalInput")
    ab = nc.dram_tensor("ab", [n_src_pad, D], BF16, kind="ExternalInput")
    bb = nc.dram_tensor("bb", [n_src_pad, D], BF16, kind="ExternalInput")
    z1o = nc.dram_tensor("z1o", [RPC, D], F32, kind="ExternalInput")
    z2o = nc.dram_tensor("z2o", [RPC, D], F32, kind="ExternalInput")
    ao = nc.dram_tensor("ao", [RPC, D], F32, kind="ExternalInput")
    bo = nc.dram_tensor("bo", [RPC, D], F32, kind="ExternalInput")
    idx_lo_d = nc.dram_tensor("idx_lo", [P, T_lo * P // 16], I16, kind="ExternalInput")
    idx_hi_d = nc.dram_tensor("idx_hi", [P, T_hi * P // 16], I16, kind="ExternalInput")
    dest_lo_d = nc.dram_tensor("dest_lo", [P, T_lo], F32, kind="ExternalInput")
    val_lo_d = nc.dram_tensor("val_lo", [P, T_lo], F32, kind="ExternalInput")
    dest_hi_d = nc.dram_tensor("dest_hi", [P, T_hi], F32, kind="ExternalInput")
    val_hi_d = nc.dram_tensor("val_hi", [P, T_hi], F32, kind="ExternalInput")
    iota_d = nc.dram_tensor("iota", [P, P], BF16, kind="ExternalInput")
    out_d = nc.dram_tensor("out", [RPC, D], F32, kind="ExternalOutput")

    zi_d = nc.dram_tensor("zi_msg", [n_src_pad, D], BF16, kind="Internal")

    one_m_alpha = float(1.0 - alpha)

    with tile.TileContext(nc) as tc:
        with (
            tc.tile_pool(name="persist", bufs=1) as pers,
            tc.tile_pool(name="psum", bufs=2, space="PSUM") as pps,
        ):
            # ---- persistent loads ----
            idx_lo_t = pers.tile([P, T_lo * P // 16], I16)
            idx_hi_t = pers.tile([P, T_hi * P // 16], I16)
            dest_lo_t = pers.tile([P, T_lo], F32)
            val_lo_t = pers.tile([P, T_lo], F32)
            dest_hi_t = pers.tile([P, T_hi], F32)
            val_hi_t = pers.tile([P, T_hi], F32)
            iota_t = pers.tile([P, P], BF16)
            nc.sync.dma_start(idx_lo_t[:], idx_lo_d[:])
            nc.sync.dma_start(idx_hi_t[:], idx_hi_d[:])
            nc.sync.dma_start(dest_lo_t[:], dest_lo_d[:])
            nc.sync.dma_start(val_lo_t[:], val_lo_d[:])
            nc.sync.dma_start(dest_hi_t[:], dest_hi_d[:])
            nc.sync.dma_start(val_hi_t[:], val_hi_d[:])
            nc.sync.dma_start(iota_t[:], iota_d[:])

            # ---- phase A2: own-shard z_i, f32, pre-scaled by (1-alpha) ----
            zio_t = pers.tile([P, NB, P], F32)  # resident residual table
            GW = next(w for w in (7, 5, 3, 2, 1) if NB % w == 0)  # blocks per group
            n_groups = NB // GW
            r4 = lambda t: t[:].rearrange("(g w p) d -> g p w d", p=P, w=GW)
            with tc.tile_pool(name="pha2", bufs=2) as pa2:
                for g in range(n_groups):
                    tz1 = pa2.tile([P, GW, P], F32, tag="tz1")
                    tz2 = pa2.tile([P, GW, P], F32, tag="tz2")
                    ta = pa2.tile([P, GW, P], F32, tag="ta")
                    tb = pa2.tile([P, GW, P], F32, tag="tb")
                    nc.sync.dma_start(tz1[:], r4(z1o)[g])
                    nc.sync.dma_start(tz2[:], r4(z2o)[g])
                    nc.sync.dma_start(ta[:], r4(ao)[g])
                    nc.sync.dma_start(tb[:], r4(bo)[g])
                    t1 = pa2.tile([P, GW, P], F32, tag="t1")
                    nc.vector.scalar_tensor_tensor(
                        out=t1[:], in0=tz1[:], scalar=one_m_alpha, in1=ta[:],
                        op0=mybir.AluOpType.mult, op1=mybir.AluOpType.mult)
                    t2 = pa2.tile([P, GW, P], F32, tag="t2")
                    nc.vector.scalar_tensor_tensor(
                        out=t2[:], in0=tz2[:], scalar=one_m_alpha, in1=tb[:],
                        op0=mybir.AluOpType.mult, op1=mybir.AluOpType.mult)
                    nc.vector.tensor_tensor(
                        out=zio_t[:, g * GW:(g + 1) * GW, :], in0=t1[:], in1=t2[:],
                        op=mybir.AluOpType.add)

            # ---- phase A: full z_i table in bf16 -> DRAM ----
            FLAT = n_src_pad * D          # 6406144
            NCH = 16
            CW = FLAT // NCH // P         # 3128
            assert FLAT == NCH * P * CW
            rf = lambda t: t[:].rearrange("n d -> (n d)").rearrange(
                "(c p f) -> c p f", c=NCH, p=P)
            zi_stores = []
            with tc.tile_pool(name="phA", bufs=2) as pa:
                for c in range(NCH):
                    s1 = pa.tile([P, CW], BF16, tag="s1")
                    s2 = pa.tile([P, CW], BF16, tag="s2")
                    sa = pa.tile([P, CW], BF16, tag="sa")
                    sb = pa.tile([P, CW], BF16, tag="sb")
                    nc.sync.dma_start(s1[:], rf(z1b)[c])
                    nc.sync.dma_start(s2[:], rf(z2b)[c])
                    nc.sync.dma_start(sa[:], rf(ab)[c])
                    nc.sync.dma_start(sb[:], rf(bb)[c])
                    u1 = pa.tile([P, CW], BF16, tag="u1")
                    nc.vector.tensor_tensor(out=u1[:], in0=s1[:], in1=sa[:],
                                            op=mybir.AluOpType.mult)
                    u2 = pa.tile([P, CW], BF16, tag="u2")
                    nc.gpsimd.tensor_tensor(out=u2[:], in0=s2[:], in1=sb[:],
                                            op=mybir.AluOpType.mult)
                    uz = pa.tile([P, CW], BF16, tag="uz")
                    nc.vector.tensor_tensor(out=uz[:], in0=u1[:], in1=u2[:],
                                            op=mybir.AluOpType.add)
                    st = nc.sync.dma_start(rf(zi_d)[c], uz[:])
                    zi_stores.append(st)

            # ---- phase B: gather + PE segment-sum + blend ----
            n_lo_calls = -(-T_lo // CALL_CH)
            n_hi_calls = -(-T_hi // CALL_CH)

            with (
                tc.tile_pool(name="mlo", bufs=2) as plo,
                tc.tile_pool(name="mhi", bufs=2) as phi,
                tc.tile_pool(name="sval", bufs=6) as psv,
                tc.tile_pool(name="pout", bufs=3) as po,
            ):
                lo_tiles = {}
                hi_tiles = {}

                def emit_call(g, which):
                    (T, pool, idx_t, src_lo, src_hi, tiles, tag) = (
                        (T_lo, plo, idx_lo_t, 0, split, lo_tiles, "mlo")
                        if which == "lo" else
                        (T_hi, phi, idx_hi_t, split, n_src_pad, hi_tiles, "mhi"))
                    t0 = g * CALL_CH
                    t1_ = min(T, t0 + CALL_CH)
                    nch = t1_ - t0
                    mt = pool.tile([P, CALL_CH, D], BF16, tag=tag)
                    inst = nc.gpsimd.dma_gather(
                        out_ap=mt[:, :nch, :],
                        in_ap=zi_d[src_lo:src_hi, :],
                        idxs_ap=idx_t[:, t0 * P // 16: t1_ * P // 16],
                        num_idxs=nch * P,
                        num_idxs_reg=nch * P,
                        elem_size=D,
                        queue_num=g % NQ if which == "lo" else (g + 2) % NQ,
                    )
                    # zi_d is written by phase A via DRAM; make the ordering explicit
                    from concourse.tile_rust import add_dep_helper
                    for st in zi_stores:
                        add_dep_helper(inst.ins, st.ins, reason="zi DRAM RAW")
                    tiles[g] = mt

                for b in range(NB):
                    acc = pps.tile([P, D], F32, tag="acc")
                    n_mm = C_lo + C_hi
                    k = 0
                    for j in range(C_lo):
                        t = b * C_lo + j
                        g, sl = divmod(t, CALL_CH)
                        if g not in lo_tiles:
                            emit_call(g, "lo")
                        sval = psv.tile([P, P], BF16, tag="sval")
                        nc.vector.tensor_scalar(
                            out=sval[:], in0=iota_t[:],
                            scalar1=dest_lo_t[:, t:t + 1],
                            scalar2=val_lo_t[:, t:t + 1],
                            op0=mybir.AluOpType.is_equal,
                            op1=mybir.AluOpType.mult)
                        nc.tensor.matmul(
                            acc[:], lhsT=sval[:], rhs=lo_tiles[g][:, sl, :],
                            start=(k == 0), stop=(k == n_mm - 1))
                        k += 1
                    for j in range(C_hi):
                        t = b * C_hi + j
                        g, sl = divmod(t, CALL_CH)
                        if g not in hi_tiles:
                            emit_call(g, "hi")
                        sval = psv.tile([P, P], BF16, tag="sval")
                        nc.vector.tensor_scalar(
                            out=sval[:], in0=iota_t[:],
                            scalar1=dest_hi_t[:, t:t + 1],
                            scalar2=val_hi_t[:, t:t + 1],
                            op0=mybir.AluOpType.is_equal,
                            op1=mybir.AluOpType.mult)
                        nc.tensor.matmul(
                            acc[:], lhsT=sval[:], rhs=hi_tiles[g][:, sl, :],
                            start=(k == 0), stop=(k == n_mm - 1))
                        k += 1

                    ot = po.tile([P, D], F32, tag="ot")
                    nc.vector.scalar_tensor_tensor(
                        out=ot[:], in0=acc[:], scalar=float(alpha),
                        in1=zio_t[:, b, :],
                        op0=mybir.AluOpType.mult, op1=mybir.AluOpType.add)
                    nc.sync.dma_start(out_d[b * P:(b + 1) * P, :], ot[:])

    nc.compile()
    return nc


def kernel(z1, z2, adj_row, adj_col, adj_val, a, b, alpha):
    global _LAST_RESULTS
    z1 = np.asarray(z1, dtype=np.float32)
    z2 = np.asarray(z2, dtype=np.float32)
    a = np.asarray(a, dtype=np.float32)
    b = np.asarray(b, dtype=np.float32)
    adj_row = np.asarray(adj_row, dtype=np.int32)
    adj_col = np.asarray(adj_col, dtype=np.int32)
    adj_val = np.asarray(adj_val, dtype=np.float32)
    alpha = float(np.asarray(alpha))

    in_maps, cfg = _host_prep(z1, z2, adj_row, adj_col, adj_val, a, b)
    nc = _build_program(cfg, alpha)

    N = cfg["N"]
    RPC = cfg["rows_per_core"]

    if _SIM:
        from concourse.bass_interp import CoreSim
        results = []
        for c in range(N_CORES):
            sim = CoreSim(nc, trace=False)
            for k, v in in_maps[c].items():
                sim.tensor(k)[:] = v
            sim.simulate()
            results.append({"out": np.array(sim.tensor("out"))})
        _LAST_RESULTS = None
    else:
        from concourse import bass_utils
        res = bass_utils.run_bass_kernel_spmd(
            nc, in_maps, core_ids=list(range(N_CORES)), trace=_TRACE,
        )
        results = res.results
        _LAST_RESULTS = res

    out = np.empty((N, D), np.float32)
    for c in range(N_CORES):
        lo = c * RPC
        hi = min(N, lo + RPC)
        if hi > lo:
            out[lo:hi] = results[c]["out"][: hi - lo]
    return out


# revision 5
# speedup vs baseline: 1.4951x; 1.4951x over previous
"""Trainium2 Bass kernel for DCRN fusion (gated combine + sparse message passing + residual).

    z_i = a*z1 + b*z2                                  [N, D]
    z_l[r] = sum_{e: row[e]==r} val[e] * z_i[col[e]]   [N, D]
    out = alpha*z_l + (1-alpha)*z_i

Sharding: dest rows are partitioned across 8 NeuronCores in 128-row blocks
(49 blocks/core). Each core computes the full z_i table in bf16 from
replicated bf16 inputs (message path), gathers source rows per edge with
SWDGE dma_gather, and performs the per-block segment-sum on the PE via
val-scaled one-hot selection matrices accumulated in PSUM. The residual
path uses exact f32 own-shard inputs.

The source table is split in two halves (int16 gather-index limit); the
kernel runs two passes (lo sources, then hi) so the z_i table production
for the hi half overlaps the lo gather/matmul pipeline. Lo-pass partial
block sums are spilled to SBUF (pre-scaled by alpha on the Scalar engine)
and recombined in the hi-pass blend.

Self-contained: all index-space preprocessing (bucketing/sorting/padding
of the edge list) is host-side numpy inside kernel().
"""

import os
import numpy as np
import ml_dtypes

import concourse.bacc as bacc
import concourse.mybir as mybir
import concourse.tile as tile
from concourse.tile_rust import add_dep_helper

P = 128
N_CORES = 8
D = 128

BF16 = mybir.dt.bfloat16
F32 = mybir.dt.float32
I16 = mybir.dt.int16

CALL_CH = 8           # gather chunks (of 128 idxs) per dma_gather call (1024-desc ring)
NQ = 4                # SWDGE queues

# exposed for the test harness
_LAST_RESULTS = None
_TRACE = os.environ.get("GNN_TRACE", "0") == "1"
_SIM = os.environ.get("GNN_SIM", "0") == "1"


def _host_prep(z1, z2, adj_row, adj_col, adj_val, a, b):
    """Bucket/sort/pad the edge list; build per-core input arrays."""
    N = z1.shape[0]
    n_blocks_total = -(-N // P)                      # 391
    blocks_per_core = -(-n_blocks_total // N_CORES)  # 49
    rows_per_core = blocks_per_core * P              # 6272
    n_src_pad = n_blocks_total * P                   # 50048
    split = n_src_pad // 2                           # 25024 (< 32768)

    bf = ml_dtypes.bfloat16
    blk = adj_row // P
    is_hi = (adj_col >= split).astype(np.int64)
    order = np.lexsort((adj_col, is_hi, blk))
    d_s = adj_row[order]
    c_s = adj_col[order]
    v_s = adj_val[order]
    h_s = is_hi[order]
    b_s = blk[order]

    key = b_s * 2 + h_s
    n_groups = n_blocks_total * 2
    cnt = np.bincount(key, minlength=n_groups)
    grp_start = np.concatenate([[0], np.cumsum(cnt)])[:-1]
    rank = np.arange(len(order)) - grp_start[key]

    cnt2 = cnt.reshape(n_blocks_total, 2)
    C_lo = max(1, int(-(-cnt2[:, 0].max() // P)))
    C_hi = max(1, int(-(-cnt2[:, 1].max() // P)))
    T_lo = blocks_per_core * C_lo
    T_hi = blocks_per_core * C_hi

    core_s = b_s // blocks_per_core
    lblk_s = b_s % blocks_per_core

    idx_lo = np.zeros((N_CORES, T_lo * P), np.int16)
    val_lo = np.zeros((N_CORES, T_lo * P), np.float32)
    slot_lo = np.zeros((N_CORES, T_lo * P), np.float32)
    idx_hi = np.zeros((N_CORES, T_hi * P), np.int16)
    val_hi = np.zeros((N_CORES, T_hi * P), np.float32)
    slot_hi = np.zeros((N_CORES, T_hi * P), np.float32)

    m = h_s == 0
    pos = lblk_s[m] * (C_lo * P) + rank[m]
    idx_lo[core_s[m], pos] = c_s[m].astype(np.int16)
    val_lo[core_s[m], pos] = v_s[m]
    slot_lo[core_s[m], pos] = (d_s[m] % P).astype(np.float32)
    m = h_s == 1
    pos = lblk_s[m] * (C_hi * P) + rank[m]
    idx_hi[core_s[m], pos] = (c_s[m] - split).astype(np.int16)
    val_hi[core_s[m], pos] = v_s[m]
    slot_hi[core_s[m], pos] = (d_s[m] % P).astype(np.float32)

    def wrap16(x):
        # [..., n] -> [..., 128, n//16]; slot i -> [i%16, i//16], replicated x8
        n = x.shape[-1]
        w = x.reshape(-1, n // 16, 16)
        w = np.swapaxes(w, -1, -2)
        return np.tile(w, (1, 8, 1))

    def meta(x, t):
        # [T*P] -> [128, T] column t = chunk t
        return np.ascontiguousarray(x.reshape(-1, t, P).swapaxes(-1, -2))

    def pad_bf(x):
        out = np.zeros((n_src_pad, D), bf)
        out[:N] = x.astype(bf)
        return out

    def own(x, c):
        out = np.zeros((rows_per_core, D), np.float32)
        lo = c * rows_per_core
        hi = min(N, lo + rows_per_core)
        if hi > lo:
            out[: hi - lo] = x[lo:hi]
        return out

    z1b, z2b, ab, bb = pad_bf(z1), pad_bf(z2), pad_bf(a), pad_bf(b)
    iota = np.tile(np.arange(P, dtype=np.float32)[None, :], (P, 1)).astype(bf)
    CMX = max(C_lo, C_hi)
    iota_c = np.ascontiguousarray(np.tile(iota[:, None, :], (1, CMX, 1)))

    idx_lo_w = wrap16(idx_lo).astype(np.int16)
    idx_hi_w = wrap16(idx_hi).astype(np.int16)

    in_maps = []
    for c in range(N_CORES):
        in_maps.append({
            "z1b": z1b, "z2b": z2b, "ab": ab, "bb": bb,
            "z1o": own(z1, c), "z2o": own(z2, c),
            "ao": own(a, c), "bo": own(b, c),
            "idx_lo": idx_lo_w[c], "idx_hi": idx_hi_w[c],
            "dest_lo": meta(slot_lo[c], T_lo), "val_lo": meta(val_lo[c], T_lo),
            "dest_hi": meta(slot_hi[c], T_hi), "val_hi": meta(val_hi[c], T_hi),
            "iota_c": iota_c,
        })

    cfg = dict(
        N=N, n_src_pad=n_src_pad, split=split,
        blocks_per_core=blocks_per_core, rows_per_core=rows_per_core,
        C_lo=C_lo, C_hi=C_hi, T_lo=T_lo, T_hi=T_hi, CMX=CMX,
    )
    return in_maps, cfg


def _build_program(cfg, alpha):
    n_src_pad = cfg["n_src_pad"]
    split = cfg["split"]
    NB = cfg["blocks_per_core"]
    RPC = cfg["rows_per_core"]
    C_lo, C_hi = cfg["C_lo"], cfg["C_hi"]
    T_lo, T_hi = cfg["T_lo"], cfg["T_hi"]
    CMX = cfg["CMX"]

    nc = bacc.Bacc("TRN2", target_bir_lowering=False, debug=False,
                   num_swdge_queues=NQ, num_devices=N_CORES)

    z1b = nc.dram_tensor("z1b", [n_src_pad, D], BF16, kind="ExternalInput")
    z2b = nc.dram_tensor("z2b", [n_src_pad, D], BF16, kind="ExternalInput")
    ab = nc.dram_tensor("ab", [n_src_pad, D], BF16, kind="ExternalInput")
    bb = nc.dram_tensor("bb", [n_src_pad, D], BF16, kind="ExternalInput")
    z1o = nc.dram_tensor("z1o", [RPC, D], F32, kind="ExternalInput")
    z2o = nc.dram_tensor("z2o", [RPC, D], F32, kind="ExternalInput")
    ao = nc.dram_tensor("ao", [RPC, D], F32, kind="ExternalInput")
    bo = nc.dram_tensor("bo", [RPC, D], F32, kind="ExternalInput")
    idx_lo_d = nc.dram_tensor("idx_lo", [P, T_lo * P // 16], I16, kind="ExternalInput")
    idx_hi_d = nc.dram_tensor("idx_hi", [P, T_hi * P // 16], I16, kind="ExternalInput")
    dest_lo_d = nc.dram_tensor("dest_lo", [P, T_lo], F32, kind="ExternalInput")
    val_lo_d = nc.dram_tensor("val_lo", [P, T_lo], F32, kind="ExternalInput")
    dest_hi_d = nc.dram_tensor("dest_hi", [P, T_hi], F32, kind="ExternalInput")
    val_hi_d = nc.dram_tensor("val_hi", [P, T_hi], F32, kind="ExternalInput")
    iota_d = nc.dram_tensor("iota_c", [P, CMX, P], BF16, kind="ExternalInput")
    out_d = nc.dram_tensor("out", [RPC, D], F32, kind="ExternalOutput")

    zi_d = nc.dram_tensor("zi_msg", [n_src_pad, D], BF16, kind="Internal")

    one_m_alpha = float(1.0 - alpha)
    AOT = mybir.AluOpType

    # phase-A flat chunking: 32 chunks of 1564 rows; lo = 0..15, hi = 16..31
    FLAT = n_src_pad * D
    NCH = 32
    CW = FLAT // NCH // P
    assert FLAT == NCH * P * CW and (NCH // 2) * CW * P == split * D

    with tile.TileContext(nc) as tc:
        with (
            tc.tile_pool(name="persist", bufs=1) as pers,
            tc.tile_pool(name="psum", bufs=2, space="PSUM") as pps,
            tc.tile_pool(name="phA", bufs=2) as pa,
            tc.tile_pool(name="mlo", bufs=3) as plo,
            tc.tile_pool(name="mhi", bufs=3) as phi,
            tc.tile_pool(name="sval", bufs=4) as psv,
            tc.tile_pool(name="pout", bufs=3) as po,
        ):
            # ---- persistent loads ----
            idx_lo_t = pers.tile([P, T_lo * P // 16], I16)
            idx_hi_t = pers.tile([P, T_hi * P // 16], I16)
            dest_lo_t = pers.tile([P, T_lo], F32)
            val_lo_t = pers.tile([P, T_lo], F32)
            dest_hi_t = pers.tile([P, T_hi], F32)
            val_hi_t = pers.tile([P, T_hi], F32)
            iota_t = pers.tile([P, CMX, P], BF16)
            nc.sync.dma_start(idx_lo_t[:], idx_lo_d[:])
            nc.sync.dma_start(idx_hi_t[:], idx_hi_d[:])
            nc.sync.dma_start(dest_lo_t[:], dest_lo_d[:])
            nc.sync.dma_start(val_lo_t[:], val_lo_d[:])
            nc.sync.dma_start(dest_hi_t[:], dest_hi_d[:])
            nc.sync.dma_start(val_hi_t[:], val_hi_d[:])
            nc.sync.dma_start(iota_t[:], iota_d[:])

            zio_t = pers.tile([P, NB, P], F32)      # (1-alpha) * z_i own rows
            part_t = pers.tile([P, NB, P], F32)     # alpha * (lo-pass partial z_l)

            # ---- phase A2: own-shard z_i residual, f32 ----
            GW = next(w for w in (7, 5, 3, 2, 1) if NB % w == 0)
            r4 = lambda t: t[:].rearrange("(g w p) d -> g p w d", p=P, w=GW)
            for g in range(NB // GW):
                tz1 = pa.tile([P, GW, P], F32, tag="tz1")
                tz2 = pa.tile([P, GW, P], F32, tag="tz2")
                ta = pa.tile([P, GW, P], F32, tag="ta")
                tb = pa.tile([P, GW, P], F32, tag="tb")
                nc.sync.dma_start(tz1[:], r4(z1o)[g])
                nc.sync.dma_start(tz2[:], r4(z2o)[g])
                nc.sync.dma_start(ta[:], r4(ao)[g])
                nc.sync.dma_start(tb[:], r4(bo)[g])
                t1 = pa.tile([P, GW, P], F32, tag="t1")
                nc.vector.scalar_tensor_tensor(
                    out=t1[:], in0=tz1[:], scalar=one_m_alpha, in1=ta[:],
                    op0=AOT.mult, op1=AOT.mult)
                t2 = pa.tile([P, GW, P], F32, tag="t2")
                nc.vector.scalar_tensor_tensor(
                    out=t2[:], in0=tz2[:], scalar=one_m_alpha, in1=tb[:],
                    op0=AOT.mult, op1=AOT.mult)
                nc.vector.tensor_tensor(
                    out=zio_t[:, g * GW:(g + 1) * GW, :], in0=t1[:], in1=t2[:],
                    op=AOT.add)

            # ---- phase A: z_i table in bf16 -> DRAM (half at a time) ----
            rf = lambda t: t[:].rearrange("n d -> (n d)").rearrange(
                "(c p f) -> c p f", c=NCH, p=P)

            def phase_a_half(c0, c1):
                stores = []
                for c in range(c0, c1):
                    s1 = pa.tile([P, CW], BF16, tag="s1")
                    s2 = pa.tile([P, CW], BF16, tag="s2")
                    sa = pa.tile([P, CW], BF16, tag="sa")
                    sb = pa.tile([P, CW], BF16, tag="sb")
                    nc.sync.dma_start(s1[:], rf(z1b)[c])
                    nc.sync.dma_start(s2[:], rf(z2b)[c])
                    nc.sync.dma_start(sa[:], rf(ab)[c])
                    nc.sync.dma_start(sb[:], rf(bb)[c])
                    u1 = pa.tile([P, CW], BF16, tag="u1")
                    nc.vector.tensor_tensor(out=u1[:], in0=s1[:], in1=sa[:],
                                            op=AOT.mult)
                    u2 = pa.tile([P, CW], BF16, tag="u2")
                    nc.vector.tensor_tensor(out=u2[:], in0=s2[:], in1=sb[:],
                                            op=AOT.mult)
                    uz = pa.tile([P, CW], BF16, tag="uz")
                    nc.vector.tensor_tensor(out=uz[:], in0=u1[:], in1=u2[:],
                                            op=AOT.add)
                    stores.append(nc.sync.dma_start(rf(zi_d)[c], uz[:]))
                return stores

            # ---- gather + segment-sum pass over one source half ----
            def pass_half(which, stores):
                (T, C, pool, idx_t, dest_t, val_t, s0, s1_) = (
                    (T_lo, C_lo, plo, idx_lo_t, dest_lo_t, val_lo_t, 0, split)
                    if which == "lo" else
                    (T_hi, C_hi, phi, idx_hi_t, dest_hi_t, val_hi_t, split, n_src_pad))
                tiles = {}

                def emit_call(g):
                    t0 = g * CALL_CH
                    t1 = min(T, t0 + CALL_CH)
                    mt = pool.tile([P, CALL_CH, D], BF16, tag="m" + which)
                    inst = nc.gpsimd.dma_gather(
                        out_ap=mt[:, :t1 - t0, :],
                        in_ap=zi_d[s0:s1_, :],
                        idxs_ap=idx_t[:, t0 * P // 16: t1 * P // 16],
                        num_idxs=(t1 - t0) * P,
                        num_idxs_reg=(t1 - t0) * P,
                        elem_size=D,
                        queue_num=g % NQ,
                    )
                    for st in stores:
                        add_dep_helper(inst.ins, st.ins, reason="zi RAW")
                    tiles[g] = mt

                for b in range(NB):
                    sval = psv.tile([P, C, P], BF16, tag="sv" + which)
                    nc.vector.tensor_tensor(
                        out=sval[:], in0=iota_t[:, :C, :],
                        in1=dest_t[:, b * C:(b + 1) * C].to_broadcast([P, C, P]),
                        op=AOT.is_equal)
                    nc.vector.tensor_tensor(
                        out=sval[:], in0=sval[:],
                        in1=val_t[:, b * C:(b + 1) * C].to_broadcast([P, C, P]),
                        op=AOT.mult)
                    acc = pps.tile([P, D], F32, tag="acc")
                    for j in range(C):
                        t = b * C + j
                        g, sl = divmod(t, CALL_CH)
                        if g not in tiles:
                            emit_call(g)
                        nc.tensor.matmul(
                            acc[:], lhsT=sval[:, j, :], rhs=tiles[g][:, sl, :],
                            start=(j == 0), stop=(j == C - 1))

                    if which == "lo":
                        # spill alpha * partial to SBUF on the Scalar engine
                        nc.scalar.activation(
                            out=part_t[:, b, :], in_=acc[:],
                            func=mybir.ActivationFunctionType.Copy,
                            scale=float(alpha))
                    else:
                        tt = po.tile([P, D], F32, tag="tt")
                        nc.vector.scalar_tensor_tensor(
                            out=tt[:], in0=acc[:], scalar=float(alpha),
                            in1=part_t[:, b, :], op0=AOT.mult, op1=AOT.add)
                        ot = po.tile([P, D], F32, tag="ot")
                        nc.vector.tensor_tensor(
                            out=ot[:], in0=tt[:], in1=zio_t[:, b, :], op=AOT.add)
                        nc.sync.dma_start(out_d[b * P:(b + 1) * P, :], ot[:])

            lo_stores = phase_a_half(0, NCH // 2)
            pass_half("lo", lo_stores)
            hi_stores = phase_a_half(NCH // 2, NCH)
            pass_half("hi", hi_stores)

    nc.compile()
    return nc


def kernel(z1, z2, adj_row, adj_col, adj_val, a, b, alpha):
    global _LAST_RESULTS
    z1 = np.asarray(z1, dtype=np.float32)
    z2 = np.asarray(z2, dtype=np.float32)
    a = np.asarray(a, dtype=np.float32)
    b = np.asarray(b, dtype=np.float32)
    adj_row = np.asarray(adj_row, dtype=np.int32)
    adj_col = np.asarray(adj_col, dtype=np.int32)
    adj_val = np.asarray(adj_val, dtype=np.float32)
    alpha = float(np.asarray(alpha))

    in_maps, cfg = _host_prep(z1, z2, adj_row, adj_col, adj_val, a, b)
    nc = _build_program(cfg, alpha)

    N = cfg["N"]
    RPC = cfg["rows_per_core"]

    if _SIM:
        from concourse.bass_interp import CoreSim
        results = []
        for c in range(N_CORES):
            sim = CoreSim(nc, trace=False)
            for k, v in in_maps[c].items():
                sim.tensor(k)[:] = v
            sim.simulate()
            results.append({"out": np.array(sim.tensor("out"))})
        _LAST_RESULTS = None
    else:
        from concourse import bass_utils
        res = bass_utils.run_bass_kernel_spmd(
            nc, in_maps, core_ids=list(range(N_CORES)), trace=_TRACE,
        )
        results = res.results
        _LAST_RESULTS = res

    out = np.empty((N, D), np.float32)
    for c in range(N_CORES):
        lo = c * RPC
        hi = min(N, lo + RPC)
        if hi > lo:
            out[lo:hi] = results[c]["out"][: hi - lo]
    return out


# revision 6
# speedup vs baseline: 1.6529x; 1.1055x over previous
"""Trainium2 Bass kernel for DCRN fusion (gated combine + sparse message passing + residual).

    z_i = a*z1 + b*z2                                  [N, D]
    z_l[r] = sum_{e: row[e]==r} val[e] * z_i[col[e]]   [N, D]
    out = alpha*z_l + (1-alpha)*z_i

Sharding: dest rows are partitioned across 8 NeuronCores in 128-row blocks
(49 blocks/core). Each core computes the full z_i table in bf16 from
replicated bf16 inputs (message path), gathers source rows per edge with
SWDGE dma_gather, and performs the per-block segment-sum on the PE via
val-scaled one-hot selection matrices accumulated in PSUM. The residual
path uses exact f32 own-shard inputs.

The source table is split in two halves (int16 gather-index limit); the
kernel runs two passes (lo sources, then hi) so the z_i table production
for the hi half overlaps the lo gather/matmul pipeline. Lo-pass partial
block sums are spilled to SBUF (pre-scaled by alpha on the Scalar engine)
and recombined in the hi-pass blend.

Self-contained: all index-space preprocessing (bucketing/sorting/padding
of the edge list) is host-side numpy inside kernel().
"""

import os
import numpy as np
import ml_dtypes

import concourse.bacc as bacc
import concourse.mybir as mybir
import concourse.tile as tile
from concourse.tile_rust import add_dep_helper

P = 128
N_CORES = 8
D = 128

BF16 = mybir.dt.bfloat16
F32 = mybir.dt.float32
I16 = mybir.dt.int16

CALL_CH = 8           # gather chunks (of 128 idxs) per dma_gather call (1024-desc ring)
NQ = 4                # SWDGE queues

# exposed for the test harness
_LAST_RESULTS = None
_TRACE = os.environ.get("GNN_TRACE", "0") == "1"
_SIM = os.environ.get("GNN_SIM", "0") == "1"


def _host_prep(z1, z2, adj_row, adj_col, adj_val, a, b):
    """Bucket/sort/pad the edge list; build per-core input arrays."""
    N = z1.shape[0]
    n_blocks_total = -(-N // P)                      # 391
    blocks_per_core = -(-n_blocks_total // N_CORES)  # 49
    rows_per_core = blocks_per_core * P              # 6272
    n_src_pad = n_blocks_total * P                   # 50048
    split = n_src_pad // 2                           # 25024 (< 32768)

    bf = ml_dtypes.bfloat16
    blk = adj_row // P
    is_hi = (adj_col >= split).astype(np.int64)
    order = np.lexsort((adj_col, is_hi, blk))
    d_s = adj_row[order]
    c_s = adj_col[order]
    v_s = adj_val[order]
    h_s = is_hi[order]
    b_s = blk[order]

    key = b_s * 2 + h_s
    n_groups = n_blocks_total * 2
    cnt = np.bincount(key, minlength=n_groups)
    grp_start = np.concatenate([[0], np.cumsum(cnt)])[:-1]
    rank = np.arange(len(order)) - grp_start[key]

    cnt2 = cnt.reshape(n_blocks_total, 2)
    C_lo = max(1, int(-(-cnt2[:, 0].max() // P)))
    C_hi = max(1, int(-(-cnt2[:, 1].max() // P)))
    T_lo = blocks_per_core * C_lo
    T_hi = blocks_per_core * C_hi

    core_s = b_s // blocks_per_core
    lblk_s = b_s % blocks_per_core

    idx_lo = np.zeros((N_CORES, T_lo * P), np.int16)
    val_lo = np.zeros((N_CORES, T_lo * P), np.float32)
    slot_lo = np.zeros((N_CORES, T_lo * P), np.float32)
    idx_hi = np.zeros((N_CORES, T_hi * P), np.int16)
    val_hi = np.zeros((N_CORES, T_hi * P), np.float32)
    slot_hi = np.zeros((N_CORES, T_hi * P), np.float32)

    m = h_s == 0
    pos = lblk_s[m] * (C_lo * P) + rank[m]
    idx_lo[core_s[m], pos] = c_s[m].astype(np.int16)
    val_lo[core_s[m], pos] = v_s[m]
    slot_lo[core_s[m], pos] = (d_s[m] % P).astype(np.float32)
    m = h_s == 1
    pos = lblk_s[m] * (C_hi * P) + rank[m]
    idx_hi[core_s[m], pos] = (c_s[m] - split).astype(np.int16)
    val_hi[core_s[m], pos] = v_s[m]
    slot_hi[core_s[m], pos] = (d_s[m] % P).astype(np.float32)

    def wrap16(x):
        # [..., n] -> [..., 128, n//16]; slot i -> [i%16, i//16], replicated x8
        n = x.shape[-1]
        w = x.reshape(-1, n // 16, 16)
        w = np.swapaxes(w, -1, -2)
        return np.tile(w, (1, 8, 1))

    def meta(x, t):
        # [T*P] -> [128, T] column t = chunk t
        return np.ascontiguousarray(x.reshape(-1, t, P).swapaxes(-1, -2))

    def pad_bf(x):
        out = np.zeros((n_src_pad, D), bf)
        out[:N] = x.astype(bf)
        return out

    def own(x, c):
        out = np.zeros((rows_per_core, D), np.float32)
        lo = c * rows_per_core
        hi = min(N, lo + rows_per_core)
        if hi > lo:
            out[: hi - lo] = x[lo:hi]
        return out

    z1b, z2b, ab, bb = pad_bf(z1), pad_bf(z2), pad_bf(a), pad_bf(b)
    iota = np.tile(np.arange(P, dtype=np.float32)[None, :], (P, 1)).astype(bf)
    CMX = max(C_lo, C_hi)
    iota_c = np.ascontiguousarray(np.tile(iota[:, None, :], (1, CMX, 1)))

    idx_lo_w = wrap16(idx_lo).astype(np.int16)
    idx_hi_w = wrap16(idx_hi).astype(np.int16)

    in_maps = []
    for c in range(N_CORES):
        in_maps.append({
            "z1b": z1b, "z2b": z2b, "ab": ab, "bb": bb,
            "z1o": own(z1, c), "z2o": own(z2, c),
            "ao": own(a, c), "bo": own(b, c),
            "idx_lo": idx_lo_w[c], "idx_hi": idx_hi_w[c],
            "dest_lo": meta(slot_lo[c], T_lo), "val_lo": meta(val_lo[c], T_lo),
            "dest_hi": meta(slot_hi[c], T_hi), "val_hi": meta(val_hi[c], T_hi),
            "iota_c": iota_c,
        })

    cfg = dict(
        N=N, n_src_pad=n_src_pad, split=split,
        blocks_per_core=blocks_per_core, rows_per_core=rows_per_core,
        C_lo=C_lo, C_hi=C_hi, T_lo=T_lo, T_hi=T_hi, CMX=CMX,
    )
    return in_maps, cfg


def _build_program(cfg, alpha):
    n_src_pad = cfg["n_src_pad"]
    split = cfg["split"]
    NB = cfg["blocks_per_core"]
    RPC = cfg["rows_per_core"]
    C_lo, C_hi = cfg["C_lo"], cfg["C_hi"]
    T_lo, T_hi = cfg["T_lo"], cfg["T_hi"]
    CMX = cfg["CMX"]

    nc = bacc.Bacc("TRN2", target_bir_lowering=False, debug=False,
                   num_swdge_queues=NQ, num_devices=N_CORES)

    z1b = nc.dram_tensor("z1b", [n_src_pad, D], BF16, kind="ExternalInput")
    z2b = nc.dram_tensor("z2b", [n_src_pad, D], BF16, kind="ExternalInput")
    ab = nc.dram_tensor("ab", [n_src_pad, D], BF16, kind="ExternalInput")
    bb = nc.dram_tensor("bb", [n_src_pad, D], BF16, kind="ExternalInput")
    z1o = nc.dram_tensor("z1o", [RPC, D], F32, kind="ExternalInput")
    z2o = nc.dram_tensor("z2o", [RPC, D], F32, kind="ExternalInput")
    ao = nc.dram_tensor("ao", [RPC, D], F32, kind="ExternalInput")
    bo = nc.dram_tensor("bo", [RPC, D], F32, kind="ExternalInput")
    idx_lo_d = nc.dram_tensor("idx_lo", [P, T_lo * P // 16], I16, kind="ExternalInput")
    idx_hi_d = nc.dram_tensor("idx_hi", [P, T_hi * P // 16], I16, kind="ExternalInput")
    dest_lo_d = nc.dram_tensor("dest_lo", [P, T_lo], F32, kind="ExternalInput")
    val_lo_d = nc.dram_tensor("val_lo", [P, T_lo], F32, kind="ExternalInput")
    dest_hi_d = nc.dram_tensor("dest_hi", [P, T_hi], F32, kind="ExternalInput")
    val_hi_d = nc.dram_tensor("val_hi", [P, T_hi], F32, kind="ExternalInput")
    iota_d = nc.dram_tensor("iota_c", [P, CMX, P], BF16, kind="ExternalInput")
    out_d = nc.dram_tensor("out", [RPC, D], F32, kind="ExternalOutput")

    zi_d = nc.dram_tensor("zi_msg", [n_src_pad, D], BF16, kind="Internal")

    one_m_alpha = float(1.0 - alpha)
    AOT = mybir.AluOpType

    # phase-A flat chunking: 32 chunks of 1564 rows; lo = 0..15, hi = 16..31
    FLAT = n_src_pad * D
    NCH = 32
    CW = FLAT // NCH // P
    assert FLAT == NCH * P * CW and (NCH // 2) * CW * P == split * D

    with tile.TileContext(nc) as tc:
        with (
            tc.tile_pool(name="persist", bufs=1) as pers,
            tc.tile_pool(name="psum", bufs=2, space="PSUM") as pps,
            tc.tile_pool(name="phA", bufs=2) as pa,
            tc.tile_pool(name="mlo", bufs=4) as plo,
            tc.tile_pool(name="mhi", bufs=4) as phi,
            tc.tile_pool(name="sval", bufs=6) as psv,
            tc.tile_pool(name="pout", bufs=3) as po,
        ):
            # ---- persistent loads ----
            idx_lo_t = pers.tile([P, T_lo * P // 16], I16)
            idx_hi_t = pers.tile([P, T_hi * P // 16], I16)
            dest_lo_t = pers.tile([P, T_lo], F32)
            val_lo_t = pers.tile([P, T_lo], F32)
            dest_hi_t = pers.tile([P, T_hi], F32)
            val_hi_t = pers.tile([P, T_hi], F32)
            iota_t = pers.tile([P, CMX, P], BF16)
            nc.sync.dma_start(idx_lo_t[:], idx_lo_d[:])
            nc.sync.dma_start(idx_hi_t[:], idx_hi_d[:])
            nc.sync.dma_start(dest_lo_t[:], dest_lo_d[:])
            nc.sync.dma_start(val_lo_t[:], val_lo_d[:])
            nc.sync.dma_start(dest_hi_t[:], dest_hi_d[:])
            nc.sync.dma_start(val_hi_t[:], val_hi_d[:])
            nc.sync.dma_start(iota_t[:], iota_d[:])

            zio_t = pers.tile([P, NB, P], F32)      # (1-alpha) * z_i own rows
            part_t = pers.tile([P, NB, P], F32)     # alpha * (lo-pass partial z_l)

            # ---- phase A2: own-shard z_i residual, f32 ----
            GW = next(w for w in (7, 5, 3, 2, 1) if NB % w == 0)
            r4 = lambda t: t[:].rearrange("(g w p) d -> g p w d", p=P, w=GW)
            for g in range(NB // GW):
                tz1 = pa.tile([P, GW, P], F32, tag="tz1")
                tz2 = pa.tile([P, GW, P], F32, tag="tz2")
                ta = pa.tile([P, GW, P], F32, tag="ta")
                tb = pa.tile([P, GW, P], F32, tag="tb")
                nc.sync.dma_start(tz1[:], r4(z1o)[g])
                nc.sync.dma_start(tz2[:], r4(z2o)[g])
                nc.sync.dma_start(ta[:], r4(ao)[g])
                nc.sync.dma_start(tb[:], r4(bo)[g])
                t1 = pa.tile([P, GW, P], F32, tag="t1")
                nc.vector.scalar_tensor_tensor(
                    out=t1[:], in0=tz1[:], scalar=one_m_alpha, in1=ta[:],
                    op0=AOT.mult, op1=AOT.mult)
                t2 = pa.tile([P, GW, P], F32, tag="t2")
                nc.vector.scalar_tensor_tensor(
                    out=t2[:], in0=tz2[:], scalar=one_m_alpha, in1=tb[:],
                    op0=AOT.mult, op1=AOT.mult)
                nc.vector.tensor_tensor(
                    out=zio_t[:, g * GW:(g + 1) * GW, :], in0=t1[:], in1=t2[:],
                    op=AOT.add)

            # ---- phase A: z_i table in bf16 -> DRAM (half at a time) ----
            rf = lambda t: t[:].rearrange("n d -> (n d)").rearrange(
                "(c p f) -> c p f", c=NCH, p=P)

            def phase_a_half(c0, c1):
                stores = []
                for c in range(c0, c1):
                    s1 = pa.tile([P, CW], BF16, tag="s1")
                    s2 = pa.tile([P, CW], BF16, tag="s2")
                    sa = pa.tile([P, CW], BF16, tag="sa")
                    sb = pa.tile([P, CW], BF16, tag="sb")
                    nc.sync.dma_start(s1[:], rf(z1b)[c])
                    nc.sync.dma_start(s2[:], rf(z2b)[c])
                    nc.sync.dma_start(sa[:], rf(ab)[c])
                    nc.sync.dma_start(sb[:], rf(bb)[c])
                    u1 = pa.tile([P, CW], BF16, tag="u1")
                    nc.vector.tensor_tensor(out=u1[:], in0=s1[:], in1=sa[:],
                                            op=AOT.mult)
                    u2 = pa.tile([P, CW], BF16, tag="u2")
                    nc.vector.tensor_tensor(out=u2[:], in0=s2[:], in1=sb[:],
                                            op=AOT.mult)
                    uz = pa.tile([P, CW], BF16, tag="uz")
                    nc.vector.tensor_tensor(out=uz[:], in0=u1[:], in1=u2[:],
                                            op=AOT.add)
                    stores.append(nc.sync.dma_start(rf(zi_d)[c], uz[:]))
                return stores

            # ---- gather + segment-sum pass over one source half ----
            def pass_half(which, stores):
                (T, C, pool, idx_t, dest_t, val_t, s0, s1_) = (
                    (T_lo, C_lo, plo, idx_lo_t, dest_lo_t, val_lo_t, 0, split)
                    if which == "lo" else
                    (T_hi, C_hi, phi, idx_hi_t, dest_hi_t, val_hi_t, split, n_src_pad))
                tiles = {}

                def emit_call(g):
                    t0 = g * CALL_CH
                    t1 = min(T, t0 + CALL_CH)
                    mt = pool.tile([P, CALL_CH, D], BF16, tag="m" + which)
                    inst = nc.gpsimd.dma_gather(
                        out_ap=mt[:, :t1 - t0, :],
                        in_ap=zi_d[s0:s1_, :],
                        idxs_ap=idx_t[:, t0 * P // 16: t1 * P // 16],
                        num_idxs=(t1 - t0) * P,
                        num_idxs_reg=(t1 - t0) * P,
                        elem_size=D,
                        queue_num=g % NQ,
                    )
                    for st in stores:
                        add_dep_helper(inst.ins, st.ins, reason="zi RAW")
                    tiles[g] = mt

                for b in range(NB):
                    sval = psv.tile([P, C, P], BF16, tag="sv" + which)
                    nc.vector.tensor_tensor(
                        out=sval[:], in0=iota_t[:, :C, :],
                        in1=dest_t[:, b * C:(b + 1) * C].to_broadcast([P, C, P]),
                        op=AOT.is_equal)
                    nc.vector.tensor_tensor(
                        out=sval[:], in0=sval[:],
                        in1=val_t[:, b * C:(b + 1) * C].to_broadcast([P, C, P]),
                        op=AOT.mult)
                    acc = pps.tile([P, D], F32, tag="acc")
                    for j in range(C):
                        t = b * C + j
                        g, sl = divmod(t, CALL_CH)
                        if g not in tiles:
                            emit_call(g)
                        nc.tensor.matmul(
                            acc[:], lhsT=sval[:, j, :], rhs=tiles[g][:, sl, :],
                            start=(j == 0), stop=(j == C - 1))

                    if which == "lo":
                        # spill alpha * partial to SBUF on the Scalar engine
                        nc.scalar.activation(
                            out=part_t[:, b, :], in_=acc[:],
                            func=mybir.ActivationFunctionType.Copy,
                            scale=float(alpha))
                    else:
                        tt = po.tile([P, D], F32, tag="tt")
                        nc.vector.scalar_tensor_tensor(
                            out=tt[:], in0=acc[:], scalar=float(alpha),
                            in1=part_t[:, b, :], op0=AOT.mult, op1=AOT.add)
                        ot = po.tile([P, D], F32, tag="ot")
                        nc.vector.tensor_tensor(
                            out=ot[:], in0=tt[:], in1=zio_t[:, b, :], op=AOT.add)
                        nc.sync.dma_start(out_d[b * P:(b + 1) * P, :], ot[:])

            lo_stores = phase_a_half(0, NCH // 2)
            hi_stores = phase_a_half(NCH // 2, NCH)
            pass_half("lo", lo_stores)
            pass_half("hi", hi_stores)

    nc.compile()
    return nc


def kernel(z1, z2, adj_row, adj_col, adj_val, a, b, alpha):
    global _LAST_RESULTS
    z1 = np.asarray(z1, dtype=np.float32)
    z2 = np.asarray(z2, dtype=np.float32)
    a = np.asarray(a, dtype=np.float32)
    b = np.asarray(b, dtype=np.float32)
    adj_row = np.asarray(adj_row, dtype=np.int32)
    adj_col = np.asarray(adj_col, dtype=np.int32)
    adj_val = np.asarray(adj_val, dtype=np.float32)
    alpha = float(np.asarray(alpha))

    in_maps, cfg = _host_prep(z1, z2, adj_row, adj_col, adj_val, a, b)
    nc = _build_program(cfg, alpha)

    N = cfg["N"]
    RPC = cfg["rows_per_core"]

    if _SIM:
        from concourse.bass_interp import CoreSim
        results = []
        for c in range(N_CORES):
            sim = CoreSim(nc, trace=False)
            for k, v in in_maps[c].items():
                sim.tensor(k)[:] = v
            sim.simulate()
            results.append({"out": np.array(sim.tensor("out"))})
        _LAST_RESULTS = None
    else:
        from concourse import bass_utils
        res = bass_utils.run_bass_kernel_spmd(
            nc, in_maps, core_ids=list(range(N_CORES)), trace=_TRACE,
        )
        results = res.results
        _LAST_RESULTS = res

    out = np.empty((N, D), np.float32)
    for c in range(N_CORES):
        lo = c * RPC
        hi = min(N, lo + RPC)
        if hi > lo:
            out[lo:hi] = results[c]["out"][: hi - lo]
    return out
